# revision 27
# baseline (speedup 1.0000x reference)
"""Trainium2 Bass kernel for nn_Absolute_attention (dense_transformer).

Math (reference algebra, simplified):
  qs[b,l,h]   = sum_hd sigmoid(X @ Wq + bq)[b,l,h,hd]
  U[t,h,:]    = [cos(phi), sin(phi)],  phi = (t + delta_h) * angle[h,:]   [L,H,128]
  (reference time basis Traw = [c+s, c-s] satisfies Traw.Traw' = 2 U.U')
  attention[b,l,t,h] = qs[b,l,h] * (U[l,h].U[t,h]) / 128
  out = LN( (qs/128 * (U @ (U^T @ V))) @ Wo + bo ),  V = X@Wv + bv
(k / Wk / attention_mask are dead code in the reference.)

The [B,L,T,H] attention tensor is never materialized: per (batch, head)
  M_h = U^T_h @ V_h   [128, 16]   (contraction over the FULL sequence)
  P_h = U_h @ M_h     [L, 16]
  A   = (qs/128) . P ;  out = LN(A @ Wo + bo)

Sharding: rows (b, l) flattened to 4096, split 8 ways -> 512 rows/core
(cores 0-3 = batch 0 = die 0, cores 4-7 = batch 1 = die 1).  Each core
computes a partial M over its 512 rows; the quad all-reduce of the
[128,256] fp16 partial is done with direct SBUF->SBUF remote DMAs to the
XOR peers (^1,^2,^3) inside the die — no collective barrier — and 3 DVE
adds.  Everything else is local.

The fast path below assumes bq=bv=bo=0 and ln_gamma=1, ln_beta=0 (true
for this problem's setup_inputs); kernel() verifies this at runtime and
falls back to a generic (collective-based) build otherwise.
"""

import numpy as np

P = 128
R = 512          # rows per core
LT = 4           # l-tiles per core
D = 256
H = 16
HD = 16
TD = 128         # 2*TIME_DIM
N_CORES = 8
B, L = 2, 2048
LN_EPS = 1e-5


# =====================================================================
# Fast path: zero biases, identity LayerNorm affine.
# =====================================================================

def build_fast():
    import concourse.bass as bass
    import concourse.mybir as mybir

    f32 = mybir.dt.float32
    f16 = mybir.dt.float16
    Act = mybir.ActivationFunctionType

    nc = bass.Bass(target_bir_lowering=False, num_devices=N_CORES)

    # ---- DRAM I/O (fp16 unless noted) ----
    d_xt = nc.dram_tensor("xt", [D, R], f16, kind="ExternalInput")        # X chunk transposed
    d_tt = [nc.dram_tensor(f"tt{i}", [P, H * TD], f16, kind="ExternalInput")
            for i in range(LT)]                                           # U chunk t-major, per t-tile
    d_td = nc.dram_tensor("td", [TD, H * R], f16, kind="ExternalInput")   # U chunk, d-major
    d_wq = nc.dram_tensor("wq", [D, D], f16, kind="ExternalInput")
    d_wv = nc.dram_tensor("wv", [D, D], f16, kind="ExternalInput")
    d_wo = nc.dram_tensor("wo", [H * 32, D], f16, kind="ExternalInput")   # K-padded Wo
    d_bb = nc.dram_tensor("bblk", [D, H * 32], f16, kind="ExternalInput")  # block-diag 1/128, M-padded
    d_out = nc.dram_tensor("out", [R, D], f16, kind="ExternalOutput")

    # collective bounce buffers (internal DRAM, fp16; Shared output is the
    # fast HBM-HBM collective path)
    mb_in = nc.dram_tensor("mb_in", [P, D], f16)
    mb_out = nc.dram_tensor("mb_out", [P, D], f16)

    from contextlib import ExitStack
    ctx = ExitStack()
    sb = lambda name, shape, dt=f32: ctx.enter_context(nc.sbuf_tensor(name, shape, dt))
    ps = lambda name, shape: ctx.enter_context(nc.psum_tensor(name, shape, f32))
    sem = lambda name: ctx.enter_context(nc.semaphore(name))

    # ---- SBUF ----
    xt_sb = sb("xt_sb", [P, 2, R], f16)     # [p, k-tile, l]
    wq_sb = sb("wq_sb", [P, 2, D], f16)
    wv_sb = sb("wv_sb", [P, 2, D], f16)
    wo_sb = sb("wo_sb", [P, 4, D], f16)
    bb_sb = sb("bb_sb", [P, 2, H * 32], f16)
    tt_sb = sb("tt_sb", [P, LT, H * TD], f16)   # [p, t-tile, (h d)]
    td_sb = sb("td_sb", [P, H, R], f16)         # [p(d), h, l]
    v_sb = sb("v_sb", [P, LT, D], f16)
    st_sb = sb("st_sb", [P, 2, R], f16)         # sigmoid(Q)^T
    qr_sb = sb("qr_sb", [P, 4, R])              # qs replicated, padded (h,32) layout
    m_sb = sb("m_sb", [P, D], f16)              # M partial (compact)
    mfull_sb = sb("mfull_sb", [P, D], f16)      # M (full) compact, CC landing
    mpad_sb = sb("mpad_sb", [P, H, 32], f16)    # M (full) padded
    macc_sb = sb("macc_sb", [P, D])             # f32 partial-M accumulator
    a_sb = sb("a_sb", [P, 4, R], f16)           # A^T padded
    y_sb = sb("y_sb", [P, LT, D], f16)          # final out tiles
    stat_sb = sb("stat_sb", [P, 6])
    mv_sb = sb("mv_sb", [P, LT, 2])
    std_sb = sb("std_sb", [P, LT])
    rstd_sb = sb("rstd_sb", [P, LT])
    eps_sb = sb("eps_sb", [P, 1])
    scr_sb = sb("scr_sb", [P, 1])               # ACT dummy target

    # ---- PSUM (8 banks) ----
    ps_v = ps("ps_v", [P, D])
    ps_m = ps("ps_m", [P, D])
    ps_q0 = ps("ps_q0", [P, R])
    ps_q1 = ps("ps_q1", [P, R])
    ps_r = [ps(f"ps_r{i}", [P, R]) for i in range(4)]

    def ps_q(a):
        return ps_q0 if a == 0 else ps_q1

    # per-t-tile partial-M regions (reusing the Qt banks, which PE only
    # writes later in program order, gated on s_macc)
    def ps_mr(ti):
        bank = ps_q0 if ti % 2 == 0 else ps_q1
        off = (ti // 2) * D
        return bank[:, off:off + D]

    def ps_out(lt):
        return [ps_v[:, :], ps_m[:, :], ps_q0[:, 0:D], ps_q1[:, 0:D]][lt]

    # ---- semaphores ----
    s_dx = sem("s_dx")        # xt dma (16 per tile-column chunk)
    s_dwv = sem("s_dwv")
    s_dtt = [sem(f"s_dtt{i}") for i in range(LT)]
    s_dwq = sem("s_dwq")      # wq
    s_dbb = sem("s_dbb")      # bblk
    s_dtd = sem("s_dtd")
    s_dwo = sem("s_dwo")
    s_do = sem("s_do")        # output stores (64)
    s_cc = sem("s_cc")        # m bounce-out (16) + mfull bounce-in (16)
    s_ccdone = sem("s_ccdone")
    s_mfull = sem("s_mfull")  # mpad scatter done
    s_pz = sem("s_pz")        # eps + mpad pad-zero memsets (2)
    s_vmm = sem("s_vmm")
    s_vcopy = sem("s_vcopy")
    s_mmm = sem("s_mmm")
    s_macc = sem("s_macc")
    s_qmm = sem("s_qmm")
    s_sig = sem("s_sig")
    s_rmm = sem("s_rmm")
    s_rcopy = sem("s_rcopy")
    s_pmm = sem("s_pmm")
    s_amul = sem("s_amul")
    s_lnc = sem("s_lnc")      # LN chain: 2 incs per l-tile (stats, aggr)
    s_std = sem("s_std")
    s_y = sem("s_y")
    s_omm = sem("s_omm")

    blk = ctx.enter_context(nc.Block())

    # ================= SP: input DMAs + output stores =================
    @blk.sync
    def _(sp):
        # M-critical prefix: xt, wv, tt tiles
        sp.dma_start(xt_sb[:], d_xt[:, :].rearrange("(a p) f -> p a f", p=P)).then_inc(s_dx, 64)
        sp.dma_start(wv_sb[:], d_wv[:, :].rearrange("(a p) f -> p a f", p=P)).then_inc(s_dwv, 16)
        for i in range(LT):
            sp.dma_start(tt_sb[:, i, :], d_tt[i][:, :]).then_inc(s_dtt[i], 16)
        # bounce the local M partial out as soon as it's ready; the HWDGE
        # queue is FIFO so everything after this line queues behind it
        sp.wait_ge(s_macc, 4)
        sp.dma_start(mb_in[:, :], m_sb[:]).then_inc(s_cc, 16)
        sp.dma_start(td_sb[:], d_td[:, :].rearrange("p (h f) -> p h f", h=H)).then_inc(s_dtd, 16)
        sp.dma_start(wq_sb[:], d_wq[:, :].rearrange("(a p) f -> p a f", p=P)).then_inc(s_dwq, 16)
        sp.dma_start(bb_sb[:], d_bb[:, :].rearrange("(a p) f -> p a f", p=P)).then_inc(s_dbb, 16)
        sp.dma_start(wo_sb[:], d_wo[:, :].rearrange("(a p) f -> p a f", p=P)).then_inc(s_dwo, 16)
        sp.wait_ge(s_ccdone, 1)
        sp.dma_start(mfull_sb[:], mb_out[:, :]).then_inc(s_cc, 16)
        for lt in range(LT):
            sp.wait_ge(s_y, lt + 1)
            sp.dma_start(d_out[lt * P:(lt + 1) * P, :], y_sb[:, lt, :]).then_inc(s_do, 16)
        sp.wait_ge(s_do, 64)

    # ================= POOL: skew-absorbing dummy CC + M AllReduce =================
    @blk.gpsimd
    def _(gp):
        gp.wait_ge(s_cc, 16)
        gp.collective_compute(
            "AllReduce", mybir.AluOpType.add,
            replica_groups=[[0, 1, 2, 3], [4, 5, 6, 7]],
            ins=[mb_in[:, :].opt()],
            outs=[mb_out[:, :].opt()],
        ).then_inc(s_ccdone, 1)

    # ================= PE: all matmuls =================
    @blk.tensor
    def _(pe):
        mm = nc.tensor.matmul
        # --- interleaved V + M-partial, t-tile by t-tile ---
        pe.wait_ge(s_dwv, 16)
        pe.wait_ge(s_dx, 64)
        for ti in range(LT):
            if ti >= 1:
                pe.wait_ge(s_vcopy, ti)
            for a in range(2):
                ins = mm(ps_v[:, :], xt_sb[:, a, ti * P:(ti + 1) * P],
                         wv_sb[:, a, :], start=(a == 0), stop=(a == 1))
            ins.then_inc(s_vmm, 1)
            pe.wait_ge(s_dtt[ti], 16)
            pe.wait_ge(s_vcopy, ti + 1)
            if ti >= 2:
                pe.wait_ge(s_macc, ti - 1)   # DVE done reading ps_mr(ti-2)
            for h in range(H):
                ins = mm(ps_mr(ti)[:, h * HD:(h + 1) * HD],
                         tt_sb[:, ti, h * TD:(h + 1) * TD],
                         v_sb[:, ti, h * HD:(h + 1) * HD],
                         start=True, stop=True)
            ins.then_inc(s_mmm, 1)
        # --- Q^T = Wq^T @ X^T ---
        pe.wait_ge(s_dwq, 16)
        pe.wait_ge(s_macc, 4)
        for a in range(2):
            for k in range(2):
                ins = mm(ps_q(a)[:, :], wq_sb[:, k, a * P:(a + 1) * P],
                         xt_sb[:, k, :], start=(k == 0), stop=(k == 1))
            ins.then_inc(s_qmm, 1)
        # --- qs_rep^T = Bblk^T @ S^T ---
        pe.wait_ge(s_dbb, 16)
        pe.wait_ge(s_sig, 2)
        for mt in range(4):
            for k in range(2):
                ins = mm(ps_r[mt][:, :], bb_sb[:, k, mt * P:(mt + 1) * P],
                         st_sb[:, k, :], start=(k == 0), stop=(k == 1))
            ins.then_inc(s_rmm, 1)
        # --- P^T (padded): per head, [32, 512] at partition 32*(h%4) of bank h//4 ---
        pe.wait_ge(s_mfull, 1)
        pe.wait_ge(s_pz, 2)
        pe.wait_ge(s_rcopy, 4)
        pe.wait_ge(s_dtd, 16)
        for bank in range(4):
            for j in range(4):
                h = bank * 4 + j
                ins = mm(ps_r[bank][32 * j:32 * j + 32, :],
                         mpad_sb[:, h, :],
                         td_sb[:, h, :],
                         start=True, stop=True,
                         tile_position=(0, 32 * j))
            ins.then_inc(s_pmm, 1)
        # --- out = A^T^T @ Wo ---
        pe.wait_ge(s_amul, 4)
        pe.wait_ge(s_dwo, 16)
        for lt in range(LT):
            for k in range(4):
                ins = mm(ps_out(lt), a_sb[:, k, lt * P:(lt + 1) * P],
                         wo_sb[:, k, :], start=(k == 0), stop=(k == 3))
            ins.then_inc(s_omm, 1)

    # ================= ACT: sigmoid + psum copies + rsqrt =================
    @blk.scalar
    def _(act):
        Act_ = Act
        # prefetch sigmoid table during the DMA head
        act.wait_ge(s_pz, 2)
        nc.scalar.activation(scr_sb[:, 0:1], eps_sb[:, 0:1],
                             Act_.Sigmoid)
        for a in range(2):
            act.wait_ge(s_qmm, a + 1)
            nc.scalar.activation(st_sb[:, a, :], ps_q(a)[:, :],
                                 Act_.Sigmoid).then_inc(s_sig, 1)
        for mt in range(4):
            act.wait_ge(s_rmm, mt + 1)
            nc.scalar.copy(qr_sb[:, mt, :], ps_r[mt][:, :]).then_inc(s_rcopy, 1)
        # prefetch sqrt table during the reduce window
        nc.scalar.activation(scr_sb[:, 0:1], eps_sb[:, 0:1], Act_.Sqrt)
        for lt in range(LT):
            act.wait_ge(s_lnc, 3 * lt + 2)
            nc.scalar.activation(std_sb[:, lt:lt + 1], mv_sb[:, lt, 1:2],
                                 Act_.Sqrt,
                                 bias=eps_sb[:, 0:1]).then_inc(s_std, 1)

    # ================= DVE: copies, reduce-combine, A-mult, LN =================
    @blk.vector
    def _(dv):
        Alu = mybir.AluOpType
        nc.vector.memset(eps_sb[:], LN_EPS).then_inc(s_pz, 1)
        nc.vector.memset(mpad_sb[:, :, HD:32], 0.0).then_inc(s_pz, 1)
        dv.wait_ge(s_dwv, 16)
        for ti in range(LT):
            dv.wait_ge(s_vmm, ti + 1)
            nc.vector.tensor_copy(v_sb[:, ti, :], ps_v[:, :]).then_inc(s_vcopy, 1)
            dv.wait_ge(s_mmm, ti + 1)
            if ti == 0:
                nc.vector.tensor_copy(macc_sb[:], ps_mr(0)).then_inc(s_macc, 1)
            elif ti < 3:
                nc.vector.tensor_tensor(macc_sb[:], macc_sb[:], ps_mr(ti),
                                        Alu.add)._wait_ge(
                    s_macc, ti).then_inc(s_macc, 1)
            else:
                nc.vector.tensor_tensor(m_sb[:], macc_sb[:], ps_mr(3),
                                        Alu.add)._wait_ge(
                    s_macc, 3).then_inc(s_macc, 1)
        dv.wait_ge(s_cc, 32)
        nc.vector.tensor_copy(
            mpad_sb[:, :, 0:HD],
            mfull_sb[:].rearrange("p (h c) -> p h c", h=H)).then_inc(s_mfull, 1)
        for bank in range(4):
            dv.wait_ge(s_pmm, bank + 1)
            nc.vector.tensor_tensor(a_sb[:, bank, :], ps_r[bank][:, :],
                                    qr_sb[:, bank, :],
                                    Alu.mult).then_inc(s_amul, 1)
        for lt in range(LT):
            c0 = 3 * lt
            dv.wait_ge(s_omm, lt + 1)
            nc.vector.bn_stats(stat_sb[:], ps_out(lt)).then_inc(s_lnc, 1)
            nc.vector.bn_aggr(mv_sb[:, lt, :], stat_sb[:])._wait_ge(
                s_lnc, c0 + 1).then_inc(s_lnc, 1)
            dv.wait_ge(s_std, lt + 1)
            nc.vector.reciprocal(rstd_sb[:, lt:lt + 1],
                                 std_sb[:, lt:lt + 1]).then_inc(s_lnc, 1)
            nc.vector.tensor_scalar(
                y_sb[:, lt, :], ps_out(lt),
                mv_sb[:, lt, 0:1], rstd_sb[:, lt:lt + 1],
                Alu.subtract,
                Alu.mult)._wait_ge(s_lnc, c0 + 3).then_inc(s_y, 1)

    ctx.close()
    nc.finalize()
    return nc


def host_prep_fast(inputs):
    """Full inputs -> list of 8 per-core input dicts (U-basis, zero-bias)."""
    X = np.asarray(inputs["tensor"], dtype=np.float32)
    Wq = np.asarray(inputs["Wq"], dtype=np.float32)
    Wv = np.asarray(inputs["Wv"], dtype=np.float32)
    Wo = np.asarray(inputs["Wo"], dtype=np.float32)
    ta = np.asarray(inputs["time_angle"], dtype=np.float32)          # [H, 64]
    delta = np.asarray(inputs["head_time_delta"], dtype=np.float32)  # [H]

    pos = np.arange(L, dtype=np.float32)
    ang = (pos[:, None, None] + delta[None, :, None]) * ta[None, :, :]  # [L,H,64]
    c, s = np.cos(ang), np.sin(ang)
    U = np.concatenate([c, s], axis=-1)                              # [L,H,128]

    wo_pad = np.zeros((H * 32, D), np.float16)
    for h in range(H):
        wo_pad[h * 32:h * 32 + HD] = Wo[h * HD:(h + 1) * HD].astype(np.float16)
    bblk = np.zeros((D, H * 32), np.float16)
    for h in range(H):
        bblk[h * HD:(h + 1) * HD, h * 32:h * 32 + HD] = np.float16(1.0 / 128.0)

    in_maps = []
    for c_id in range(N_CORES):
        b, j = divmod(c_id, 4)
        sl = slice(j * R, (j + 1) * R)
        chunk = X[b, sl]                                  # [512, 256]
        uchunk = U[sl]                                    # [512, H, 128]
        tt = uchunk.reshape(R, H * TD).astype(np.float16)
        m = {
            "xt": np.ascontiguousarray(chunk.T).astype(np.float16),
            "td": np.ascontiguousarray(
                uchunk.transpose(2, 1, 0).reshape(TD, H * R)).astype(np.float16),
            "wq": Wq.astype(np.float16), "wv": Wv.astype(np.float16),
            "wo": wo_pad, "bblk": bblk,
        }
        for i in range(LT):
            m[f"tt{i}"] = np.ascontiguousarray(tt[i * P:(i + 1) * P])
        in_maps.append(m)
    return in_maps


# =====================================================================
# Generic fallback (original collective-based kernel, handles biases).
# =====================================================================

def build_generic(use_gb=True):
    import concourse.bass as bass
    import concourse.mybir as mybir

    f32 = mybir.dt.float32
    f16 = mybir.dt.float16

    nc = bass.Bass(target_bir_lowering=False, num_devices=N_CORES)

    d_xt = nc.dram_tensor("xt", [D, R], f16, kind="ExternalInput")
    d_tt = [nc.dram_tensor(f"tt{i}", [P, H * TD], f16, kind="ExternalInput")
            for i in range(LT)]
    d_td = nc.dram_tensor("td", [TD, H * R], f16, kind="ExternalInput")
    d_wq = nc.dram_tensor("wq", [D, D], f16, kind="ExternalInput")
    d_wv = nc.dram_tensor("wv", [D, D], f16, kind="ExternalInput")
    d_wo = nc.dram_tensor("wo", [H * 32, D], f16, kind="ExternalInput")
    d_bb = nc.dram_tensor("bblk", [D, H * 32], f16, kind="ExternalInput")
    d_bq = nc.dram_tensor("bq2", [P, 2], f32, kind="ExternalInput")
    d_bvr = nc.dram_tensor("bvr", [P, D], f32, kind="ExternalInput")
    d_bor = nc.dram_tensor("bor", [P, D], f32, kind="ExternalInput")
    d_gr = nc.dram_tensor("gr", [P, D], f32, kind="ExternalInput")
    d_br = nc.dram_tensor("br", [P, D], f32, kind="ExternalInput")
    d_out = nc.dram_tensor("out", [R, D], f32, kind="ExternalOutput")

    mb_in = nc.dram_tensor("mb_in", [P, D], f16)
    mb_out = nc.dram_tensor("mb_out", [P, D], f16)

    from contextlib import ExitStack
    ctx = ExitStack()
    sb = lambda name, shape, dt=f32: ctx.enter_context(nc.sbuf_tensor(name, shape, dt))
    ps = lambda name, shape: ctx.enter_context(nc.psum_tensor(name, shape, f32))
    sem = lambda name: ctx.enter_context(nc.semaphore(name))

    xt_sb = sb("xt_sb", [P, 2, R], f16)
    wq_sb = sb("wq_sb", [P, 2, D], f16)
    wv_sb = sb("wv_sb", [P, 2, D], f16)
    wo_sb = sb("wo_sb", [P, 4, D], f16)
    bb_sb = sb("bb_sb", [P, 2, H * 32], f16)
    bq_sb = sb("bq_sb", [P, 2])
    bvr_sb = sb("bvr_sb", [P, D])
    bor_sb = sb("bor_sb", [P, D])
    gr_sb = sb("gr_sb", [P, D])
    br_sb = sb("br_sb", [P, D])
    tt_sb = sb("tt_sb", [P, LT, H * TD], f16)
    td_sb = sb("td_sb", [P, H, R], f16)
    v_sb = sb("v_sb", [P, LT, D], f16)
    st_sb = sb("st_sb", [P, 2, R], f16)
    qr_sb = sb("qr_sb", [P, 4, R])
    mpad_sb = sb("mpad_sb", [P, H, 32], f16)
    m_sb = sb("m_sb", [P, D], f16)
    macc_sb = sb("macc_sb", [P, D])
    a_sb = sb("a_sb", [P, 4, R], f16)
    y_sb = sb("y_sb", [P, LT, D])
    ytmp_sb = sb("ytmp_sb", [P, LT, D])
    stat_sb = sb("stat_sb", [P, 6])
    mv_sb = sb("mv_sb", [P, LT, 2])
    std_sb = sb("std_sb", [P, LT])
    rstd_sb = sb("rstd_sb", [P, LT])
    eps_sb = sb("eps_sb", [P, 1])

    ps_v = ps("ps_v", [P, D])
    ps_m = ps("ps_m", [P, D])
    ps_q0 = ps("ps_q0", [P, R])
    ps_q1 = ps("ps_q1", [P, R])
    ps_r = [ps(f"ps_r{i}", [P, R]) for i in range(4)]

    def ps_q(a):
        return ps_q0 if a == 0 else ps_q1

    def ps_mr(ti):
        bank = ps_q0 if ti % 2 == 0 else ps_q1
        off = (ti // 2) * D
        return bank[:, off:off + D]

    def ps_out(lt):
        return [ps_v[:, :], ps_m[:, :], ps_q0[:, 0:D], ps_q1[:, 0:D]][lt]

    s_dx = sem("s_dx")
    s_dwv = sem("s_dwv")
    s_dtt = [sem(f"s_dtt{i}") for i in range(LT)]
    s_dwq = sem("s_dwq")
    s_dtd = sem("s_dtd")
    s_dwo = sem("s_dwo")
    s_do = sem("s_do")
    s_cc = sem("s_cc")
    s_ccdone = sem("s_ccdone")
    s_eps = sem("s_eps")
    s_vmm = sem("s_vmm")
    s_vcopy = sem("s_vcopy")
    s_mmm = sem("s_mmm")
    s_macc = sem("s_macc")
    s_mcopy = sem("s_mcopy")
    s_qmm = sem("s_qmm")
    s_sig = sem("s_sig")
    s_rmm = sem("s_rmm")
    s_rcopy = sem("s_rcopy")
    s_pmm = sem("s_pmm")
    s_amul = sem("s_amul")
    s_lnc = sem("s_lnc")
    s_std = sem("s_std")
    s_y = sem("s_y")
    s_omm = sem("s_omm")

    blk = ctx.enter_context(nc.Block())

    @blk.sync
    def _(sp):
        sp.dma_start(xt_sb[:], d_xt[:, :].rearrange("(a p) f -> p a f", p=P)).then_inc(s_dx, 16)
        sp.dma_start(wv_sb[:], d_wv[:, :].rearrange("(a p) f -> p a f", p=P)).then_inc(s_dwv, 16)
        sp.dma_start(bvr_sb[:], d_bvr[:, :]).then_inc(s_dwv, 16)
        for i in range(LT):
            sp.dma_start(tt_sb[:, i, :], d_tt[i][:, :]).then_inc(s_dtt[i], 16)
        sp.wait_ge(s_macc, 4)
        sp.dma_start(mb_in[:, :], m_sb[:]).then_inc(s_cc, 16)
        sp.dma_start(wq_sb[:], d_wq[:, :].rearrange("(a p) f -> p a f", p=P)).then_inc(s_dwq, 16)
        sp.dma_start(bq_sb[:], d_bq[:, :]).then_inc(s_dwq, 16)
        sp.dma_start(bb_sb[:], d_bb[:, :].rearrange("(a p) f -> p a f", p=P)).then_inc(s_dwq, 16)
        sp.dma_start(td_sb[:], d_td[:, :].rearrange("p (h f) -> p h f", h=H)).then_inc(s_dtd, 16)
        sp.dma_start(wo_sb[:], d_wo[:, :].rearrange("(a p) f -> p a f", p=P)).then_inc(s_dwo, 16)
        sp.dma_start(bor_sb[:], d_bor[:, :]).then_inc(s_dwo, 16)
        sp.dma_start(gr_sb[:], d_gr[:, :]).then_inc(s_dwo, 16)
        sp.dma_start(br_sb[:], d_br[:, :]).then_inc(s_dwo, 16)
        sp.wait_ge(s_ccdone, 1)
        sp.dma_start(
            mpad_sb[:, :, 0:HD],
            mb_out[:, :].rearrange("p (h c) -> p h c", h=H),
        ).then_inc(s_cc, 16)
        for lt in range(LT):
            sp.wait_ge(s_y, lt + 1)
            sp.dma_start(d_out[lt * P:(lt + 1) * P, :], y_sb[:, lt, :]).then_inc(s_do, 16)
        sp.wait_ge(s_do, 64)

    @blk.gpsimd
    def _(gp):
        import concourse.mybir as mybir
        gp.memset(eps_sb[:], LN_EPS).then_inc(s_eps, 1)
        gp.memset(mpad_sb[:, :, HD:32], 0.0).then_inc(s_eps, 1)
        gp.wait_ge(s_cc, 16)
        gp.collective_compute(
            "AllReduce", mybir.AluOpType.add,
            replica_groups=[[0, 1, 2, 3], [4, 5, 6, 7]],
            ins=[mb_in[:, :].opt()],
            outs=[mb_out[:, :].opt()],
        ).then_inc(s_ccdone, 1)

    @blk.tensor
    def _(pe):
        mm = nc.tensor.matmul
        pe.wait_ge(s_dx, 16)
        pe.wait_ge(s_dwv, 32)
        for ti in range(LT):
            if ti >= 1:
                pe.wait_ge(s_vcopy, ti)
            for a in range(2):
                ins = mm(ps_v[:, :], xt_sb[:, a, ti * P:(ti + 1) * P],
                         wv_sb[:, a, :], start=(a == 0), stop=(a == 1))
            ins.then_inc(s_vmm, 1)
            pe.wait_ge(s_dtt[ti], 16)
            pe.wait_ge(s_vcopy, ti + 1)
            if ti >= 2:
                pe.wait_ge(s_macc, ti - 1)
            for h in range(H):
                ins = mm(ps_mr(ti)[:, h * HD:(h + 1) * HD],
                         tt_sb[:, ti, h * TD:(h + 1) * TD],
                         v_sb[:, ti, h * HD:(h + 1) * HD],
                         start=True, stop=True)
            ins.then_inc(s_mmm, 1)
        pe.wait_ge(s_dwq, 48)
        pe.wait_ge(s_macc, 4)
        for a in range(2):
            for k in range(2):
                ins = mm(ps_q(a)[:, :], wq_sb[:, k, a * P:(a + 1) * P],
                         xt_sb[:, k, :], start=(k == 0), stop=(k == 1))
            ins.then_inc(s_qmm, 1)
        pe.wait_ge(s_sig, 2)
        for mt in range(4):
            for k in range(2):
                ins = mm(ps_r[mt][:, :], bb_sb[:, k, mt * P:(mt + 1) * P],
                         st_sb[:, k, :], start=(k == 0), stop=(k == 1))
            ins.then_inc(s_rmm, 1)
        pe.wait_ge(s_cc, 32)
        pe.wait_ge(s_eps, 2)
        pe.wait_ge(s_rcopy, 4)
        pe.wait_ge(s_dtd, 16)
        for bank in range(4):
            for j in range(4):
                h = bank * 4 + j
                ins = mm(ps_r[bank][32 * j:32 * j + 32, :],
                         mpad_sb[:, h, :],
                         td_sb[:, h, :],
                         start=True, stop=True,
                         tile_position=(0, 32 * j))
            ins.then_inc(s_pmm, 1)
        pe.wait_ge(s_amul, 4)
        pe.wait_ge(s_dwo, 64)
        for lt in range(LT):
            for k in range(4):
                ins = mm(ps_out(lt), a_sb[:, k, lt * P:(lt + 1) * P],
                         wo_sb[:, k, :], start=(k == 0), stop=(k == 3))
            ins.then_inc(s_omm, 1)

    @blk.scalar
    def _(act):
        import concourse.mybir as mybir
        for a in range(2):
            act.wait_ge(s_qmm, a + 1)
            nc.scalar.activation(st_sb[:, a, :], ps_q(a)[:, :],
                                 mybir.ActivationFunctionType.Sigmoid,
                                 bias=bq_sb[:, a:a + 1]).then_inc(s_sig, 1)
        for mt in range(4):
            act.wait_ge(s_rmm, mt + 1)
            nc.scalar.copy(qr_sb[:, mt, :], ps_r[mt][:, :]).then_inc(s_rcopy, 1)
        act.wait_ge(s_eps, 1)
        for lt in range(LT):
            act.wait_ge(s_lnc, (6 if use_gb else 4) * lt + 3)
            nc.scalar.activation(std_sb[:, lt:lt + 1], mv_sb[:, lt, 1:2],
                                 mybir.ActivationFunctionType.Sqrt,
                                 bias=eps_sb[:, 0:1]).then_inc(s_std, 1)

    @blk.vector
    def _(dv):
        import concourse.mybir as mybir
        dv.wait_ge(s_dwv, 32)
        for ti in range(LT):
            dv.wait_ge(s_vmm, ti + 1)
            nc.vector.tensor_tensor(v_sb[:, ti, :], ps_v[:, :], bvr_sb[:],
                                    mybir.AluOpType.add).then_inc(s_vcopy, 1)
            dv.wait_ge(s_mmm, ti + 1)
            if ti == 0:
                nc.vector.tensor_copy(macc_sb[:], ps_mr(0)).then_inc(s_macc, 1)
            elif ti < 3:
                nc.vector.tensor_tensor(macc_sb[:], macc_sb[:], ps_mr(ti),
                                        mybir.AluOpType.add)._wait_ge(
                    s_macc, ti).then_inc(s_macc, 1)
            else:
                nc.vector.tensor_tensor(m_sb[:], macc_sb[:], ps_mr(3),
                                        mybir.AluOpType.add)._wait_ge(
                    s_macc, 3).then_inc(s_macc, 1)
        for bank in range(4):
            dv.wait_ge(s_pmm, bank + 1)
            nc.vector.tensor_tensor(a_sb[:, bank, :], ps_r[bank][:, :],
                                    qr_sb[:, bank, :],
                                    mybir.AluOpType.mult).then_inc(s_amul, 1)
        dv.wait_ge(s_dwo, 64)
        for lt in range(LT):
            c0 = (6 if use_gb else 4) * lt
            dv.wait_ge(s_omm, lt + 1)
            nc.vector.tensor_tensor(ytmp_sb[:, lt, :], ps_out(lt), bor_sb[:],
                                    mybir.AluOpType.add).then_inc(s_lnc, 1)
            nc.vector.bn_stats(stat_sb[:], ytmp_sb[:, lt, :])._wait_ge(
                s_lnc, c0 + 1).then_inc(s_lnc, 1)
            nc.vector.bn_aggr(mv_sb[:, lt, :], stat_sb[:])._wait_ge(
                s_lnc, c0 + 2).then_inc(s_lnc, 1)
            dv.wait_ge(s_std, lt + 1)
            nc.vector.reciprocal(rstd_sb[:, lt:lt + 1],
                                 std_sb[:, lt:lt + 1]).then_inc(s_lnc, 1)
            ts_ins = nc.vector.tensor_scalar(
                y_sb[:, lt, :], ytmp_sb[:, lt, :],
                mv_sb[:, lt, 0:1], rstd_sb[:, lt:lt + 1],
                mybir.AluOpType.subtract,
                mybir.AluOpType.mult)._wait_ge(s_lnc, c0 + 4)
            if use_gb:
                ts_ins.then_inc(s_lnc, 1)
                nc.vector.tensor_tensor(y_sb[:, lt, :], y_sb[:, lt, :], gr_sb[:],
                                        mybir.AluOpType.mult)._wait_ge(
                    s_lnc, c0 + 5).then_inc(s_lnc, 1)
                nc.vector.tensor_tensor(y_sb[:, lt, :], y_sb[:, lt, :], br_sb[:],
                                        mybir.AluOpType.add)._wait_ge(
                    s_lnc, c0 + 6).then_inc(s_y, 1)
            else:
                ts_ins.then_inc(s_y, 1)

    ctx.close()
    nc.finalize()
    return nc


def host_prep_generic(inputs):
    X = np.asarray(inputs["tensor"], dtype=np.float32)
    Wq = np.asarray(inputs["Wq"], dtype=np.float32)
    bq = np.asarray(inputs["bq"], dtype=np.float32)
    Wv = np.asarray(inputs["Wv"], dtype=np.float32)
    bv = np.asarray(inputs["bv"], dtype=np.float32)
    Wo = np.asarray(inputs["Wo"], dtype=np.float32)
    bo = np.asarray(inputs["bo"], dtype=np.float32)
    ta = np.asarray(inputs["time_angle"], dtype=np.float32)
    delta = np.asarray(inputs["head_time_delta"], dtype=np.float32)
    gam = np.asarray(inputs["ln_gamma"], dtype=np.float32)
    bet = np.asarray(inputs["ln_beta"], dtype=np.float32)

    pos = np.arange(L, dtype=np.float32)
    ang = (pos[:, None, None] + delta[None, :, None]) * ta[None, :, :]
    c, s = np.cos(ang), np.sin(ang)
    Traw = np.concatenate([c + s, c - s], axis=-1)

    wo_pad = np.zeros((H * 32, D), np.float16)
    for h in range(H):
        wo_pad[h * 32:h * 32 + HD] = Wo[h * HD:(h + 1) * HD].astype(np.float16)
    bblk = np.zeros((D, H * 32), np.float16)
    for h in range(H):
        bblk[h * HD:(h + 1) * HD, h * 32:h * 32 + HD] = np.float16(1.0 / 256.0)

    bq2 = np.ascontiguousarray(bq.reshape(2, P).T)
    bvr = np.broadcast_to(bv, (P, D)).astype(np.float32).copy()
    bor = np.broadcast_to(bo, (P, D)).astype(np.float32).copy()
    gr = np.broadcast_to(gam, (P, D)).astype(np.float32).copy()
    br = np.broadcast_to(bet, (P, D)).astype(np.float32).copy()

    in_maps = []
    for c_id in range(N_CORES):
        b, j = divmod(c_id, 4)
        sl = slice(j * R, (j + 1) * R)
        chunk = X[b, sl]
        tchunk = Traw[sl]
        tt = tchunk.reshape(R, H * TD).astype(np.float16)
        m = {
            "xt": np.ascontiguousarray(chunk.T).astype(np.float16),
            "td": np.ascontiguousarray(
                tchunk.transpose(2, 1, 0).reshape(TD, H * R)).astype(np.float16),
            "wq": Wq.astype(np.float16), "wv": Wv.astype(np.float16),
            "wo": wo_pad, "bblk": bblk, "bq2": bq2,
            "bvr": bvr, "bor": bor, "gr": gr, "br": br,
        }
        for i in range(LT):
            m[f"tt{i}"] = np.ascontiguousarray(tt[i * P:(i + 1) * P])
        in_maps.append(m)
    return in_maps


def gather(results):
    out = np.zeros((B, L, D), np.float32)
    for c_id in range(N_CORES):
        b, j = divmod(c_id, 4)
        out[b, j * R:(j + 1) * R] = results[c_id]["out"]
    return out


_CACHED = {}


def _is_fast_case(inputs):
    return (
        np.all(np.asarray(inputs["bq"]) == 0.0)
        and np.all(np.asarray(inputs["bv"]) == 0.0)
        and np.all(np.asarray(inputs["bo"]) == 0.0)
        and np.all(np.asarray(inputs["ln_gamma"]) == 1.0)
        and np.all(np.asarray(inputs["ln_beta"]) == 0.0)
    )


def kernel(**inputs):
    from concourse.bass_utils import run_bass_kernel_spmd
    if _is_fast_case(inputs):
        if "fast" not in _CACHED:
            _CACHED["fast"] = build_fast()
        nc = _CACHED["fast"]
        in_maps = host_prep_fast(inputs)
    else:
        use_gb = not (
            np.all(np.asarray(inputs["ln_gamma"]) == 1.0)
            and np.all(np.asarray(inputs["ln_beta"]) == 0.0)
        )
        key = ("gen", use_gb)
        if key not in _CACHED:
            _CACHED[key] = build_generic(use_gb)
        nc = _CACHED[key]
        in_maps = host_prep_generic(inputs)
    res = run_bass_kernel_spmd(nc, in_maps, core_ids=list(range(N_CORES)))
    return gather(res.results)
